# revision 1
# baseline (speedup 1.0000x reference)
"""Trainium2 Bass kernel for out = exp(-M) @ x.

M: [16384, 16384] fp32, x: [16384, 128] fp32 -> out: [16384, 128] fp32.

Sharding: row-shard M and out over 8 cores (2048 rows each), x replicated.

Per-core pipeline (all engines overlapped, DMA-bound at ~128 MiB HBM reads):
  DMA   : M tiles [128, 4096] fp32, natural layout (16 KiB contiguous rows),
          issue alternates SP / ACT sequencers to spread HWDGE setup cost
  ACT   : e = exp(-M_tile) fused fp32 -> bf16 (free affine scale=-1)
  PE    : transpose e chunks [128m, 128k] -> PSUM [128k, 128m] (bf16)
  DVE   : evacuate PSUM -> SBUF rhs tiles [128k, 512m]
  PE    : out.T[f, m] += x[kchunk].T @ rhs   (x stationary bf16, fp32 PSUM acc)
  PE/DVE: final [f, m] -> [m, f] transpose, store via SWDGE
"""

import sys

sys.path.insert(0, "/opt/trn_rl_repo")

import numpy as np

import concourse.bass as bass  # noqa: F401  (engine namespaces live on nc)
import concourse.mybir as mybir
import concourse.tile as tile
from concourse import bacc
from concourse.bass_utils import run_bass_kernel_spmd
from concourse.masks import make_identity

N = 16384  # M is [N, N]
D = 128  # x is [N, D]
N_CORES = 8
M_ROWS = N // N_CORES  # 2048 rows of M / out per core

F32 = mybir.dt.float32
BF16 = mybir.dt.bfloat16
EXP = mybir.ActivationFunctionType.Exp

# geometry
M_SUPER = 512  # output rows accumulated per PSUM bank
N_SUPERS = M_ROWS // M_SUPER  # 4
import os as _os
K_WIN = int(_os.environ.get("KWIN", "4096"))  # contraction window per M DMA tile
N_WINS = N // K_WIN  # 4
M_SUBS = M_SUPER // 128  # 4 m-subtiles per super
KC_PER_WIN = K_WIN // 128  # 32 k-chunks per window
N_KCHUNKS = N // 128  # 128 total k-chunks
X_STAGE = 4096  # x staging chunk (fp32) free-dim


import os

BUFS_M = int(os.environ.get("BUFS_M", "5"))
BUFS_E = int(os.environ.get("BUFS_E", "7"))
SPLIT_DMA = int(os.environ.get("SPLIT_DMA", "1"))
BUFS_PT = int(os.environ.get("BUFS_PT", "5"))
BUFS_RHS = int(os.environ.get("BUFS_RHS", "6"))
KWIN_ENV = int(os.environ.get("KWIN", "4096"))


def build_kernel(repeats=1, mode="full"):
    nc = bacc.Bacc("TRN2", target_bir_lowering=False, debug=False)
    m_ap = nc.dram_tensor("m_shard", [M_ROWS, N], F32, kind="ExternalInput").ap()
    x_ap = nc.dram_tensor("x", [N, D], F32, kind="ExternalInput").ap()
    out_ap = nc.dram_tensor("out", [M_ROWS, D], F32, kind="ExternalOutput").ap()

    from contextlib import ExitStack

    with tile.TileContext(nc) as tc, ExitStack() as ctx:
        if repeats > 1:
            ctx.enter_context(tc.For_i(0, repeats, 1))
        consts = ctx.enter_context(tc.tile_pool(name="consts", bufs=1))
        ident_bf = consts.tile([128, 128], BF16)
        make_identity(nc, ident_bf[:])
        ident_f32 = consts.tile([128, 128], F32)
        make_identity(nc, ident_f32[:])

        # x resident in SBUF as bf16, chunk c at xbf[:, c*128:(c+1)*128]
        # (partition = k within chunk, free = feature).  Loaded via SWDGE
        # (Pool) with a strided AP, converted fp32->bf16 on DVE.
        xbf_t = consts.tile([128, N_KCHUNKS * D], BF16)
        with tc.tile_pool(name="xstage", bufs=4) as xstage:
            for c in range(N_KCHUNKS):
                xs = xstage.tile([128, D], F32)
                x_eng = nc.gpsimd if os.environ.get("X_GPSIMD") else nc.sync
                x_eng.dma_start(out=xs[:], in_=x_ap[c * 128 : (c + 1) * 128, :])
                nc.vector.tensor_copy(xbf_t[:, c * D : (c + 1) * D], xs[:])

        m_pool = ctx.enter_context(tc.tile_pool(name="m", bufs=BUFS_M))
        e_pool = ctx.enter_context(tc.tile_pool(name="e", bufs=BUFS_E))
        rhs_pool = ctx.enter_context(tc.tile_pool(name="rhs", bufs=BUFS_RHS))
        outT_pool = ctx.enter_context(tc.tile_pool(name="outT", bufs=2))
        outf_pool = ctx.enter_context(tc.tile_pool(name="outf", bufs=2))
        pt_pool = ctx.enter_context(tc.tile_pool(name="pt", bufs=BUFS_PT, space="PSUM"))
        pout_pool = ctx.enter_context(tc.tile_pool(name="pout", bufs=2, space="PSUM"))
        pfin_pool = ctx.enter_context(tc.tile_pool(name="pfin", bufs=int(os.environ.get("BUFS_PFIN", "1")), space="PSUM"))

        for ms in range(N_SUPERS):
            pout = (
                pout_pool.tile([128, M_SUPER], F32, name="pout", tag="pout")
                if mode not in ("mem", "dma")
                else None
            )
            outT_mem = (
                outT_pool.tile([128, M_SUPER], F32, name="outT", tag="outT")
                if mode in ("mem", "dma")
                else None
            )
            for kw in range(N_WINS):
                ebf = []
                for j in range(M_SUBS):
                    mt = m_pool.tile([128, K_WIN], F32)
                    r0 = ms * M_SUPER + j * 128
                    c0 = kw * K_WIN
                    w = K_WIN // SPLIT_DMA
                    for s in range(SPLIT_DMA):
                        mix = os.environ.get("DMA_MIX", "")
                        idx = j * SPLIT_DMA + s
                        if mix == "hwsw":
                            dma_eng = nc.sync if idx % 2 == 0 else nc.gpsimd
                        elif mix == "3way":
                            dma_eng = (nc.sync, nc.scalar, nc.gpsimd)[idx % 3]
                        elif mix == "sync":
                            dma_eng = nc.sync
                        else:
                            dma_eng = nc.sync if idx % 2 == 0 else nc.scalar
                        dma_eng.dma_start(
                            out=mt[:, s * w : (s + 1) * w],
                            in_=m_ap[r0 : r0 + 128, c0 + s * w : c0 + (s + 1) * w],
                        )
                    if mode == "dma":
                        nc.vector.tensor_copy(
                            outT_mem[:, j * 128 : (j + 1) * 128], mt[:, 0:128]
                        )
                        continue
                    e = e_pool.tile([128, K_WIN], BF16)
                    nc.scalar.activation(e[:], mt[:], EXP, scale=-1.0)
                    ebf.append(e)
                if mode == "dma":
                    continue
                if mode == "mem":
                    # probe: DMA + exp only; consume every e tile cheaply
                    for j in range(M_SUBS):
                        nc.vector.tensor_copy(
                            outT_mem[:, j * 128 : (j + 1) * 128], ebf[j][:, 0:128]
                        )
                    continue
                for kc in range(KC_PER_WIN):
                    kg = kw * KC_PER_WIN + kc
                    if mode == "noT":
                        # probe: skip transposes+copies; feed MM junk rhs
                        off = min(kc * 128, K_WIN - M_SUPER)
                        nc.tensor.matmul(
                            pout[:],
                            lhsT=xbf_t[:, kg * D : (kg + 1) * D],
                            rhs=ebf[0][:, off : off + M_SUPER],
                            start=(kg == 0),
                            stop=(kg == N_KCHUNKS - 1),
                        )
                        continue
                    pt = pt_pool.tile([128, M_SUPER], BF16)
                    for j in range(M_SUBS):
                        nc.tensor.transpose(
                            pt[:, j * 128 : (j + 1) * 128],
                            ebf[j][:, kc * 128 : (kc + 1) * 128],
                            ident_bf[:],
                        )
                    rhs = rhs_pool.tile([128, M_SUPER], BF16)
                    nc.vector.tensor_copy(rhs[:], pt[:])
                    nc.tensor.matmul(
                        pout[:],
                        lhsT=xbf_t[:, kg * D : (kg + 1) * D],
                        rhs=rhs[:],
                        start=(kg == 0),
                        stop=(kg == N_KCHUNKS - 1),
                    )
            # evacuate out.T [f, m] and transpose to [m, f]
            if mode in ("mem", "dma"):
                outT = outT_mem
            else:
                outT = outT_pool.tile([128, M_SUPER], F32)
                nc.vector.tensor_copy(outT[:], pout[:])
            for j in range(M_SUBS):
                pf = pfin_pool.tile([128, D], F32)
                nc.tensor.transpose(
                    pf[:], outT[:, j * 128 : (j + 1) * 128], ident_f32[:]
                )
                of = outf_pool.tile([128, D], F32)
                nc.vector.tensor_copy(of[:], pf[:])
                r0 = ms * M_SUPER + j * 128
                o_eng = nc.gpsimd if os.environ.get("X_GPSIMD") else nc.scalar
                o_eng.dma_start(out=out_ap[r0 : r0 + 128, :], in_=of[:])

    nc.compile()
    return nc


_NC_CACHE = None


def _get_nc():
    global _NC_CACHE
    if _NC_CACHE is None:
        _NC_CACHE = build_kernel()
    return _NC_CACHE


def _run_on_device(M, x):
    nc = _get_nc()
    in_maps = [
        {"m_shard": M[c * M_ROWS : (c + 1) * M_ROWS], "x": x} for c in range(N_CORES)
    ]
    res = run_bass_kernel_spmd(nc, in_maps, list(range(N_CORES)))
    return np.concatenate([res.results[c]["out"] for c in range(N_CORES)], axis=0)


def _run_in_subprocess(M, x):
    """Retry path: a fresh process gets a fresh NRT/axon session, which
    recovers from the occasional NRT_EXEC_UNIT_UNRECOVERABLE flake."""
    import os, subprocess, tempfile

    d = tempfile.mkdtemp(prefix="bassk_")
    np.save(os.path.join(d, "M.npy"), M)
    np.save(os.path.join(d, "x.npy"), x)
    here = os.path.dirname(os.path.abspath(__file__))
    code = (
        "import sys, numpy as np\n"
        f"sys.path.insert(0, {here!r})\n"
        "import kernel\n"
        f"M = np.load({os.path.join(d, 'M.npy')!r})\n"
        f"x = np.load({os.path.join(d, 'x.npy')!r})\n"
        "out = kernel._run_on_device(M, x)\n"
        f"np.save({os.path.join(d, 'out.npy')!r}, out)\n"
    )
    subprocess.run([sys.executable, "-c", code], check=True, timeout=1200)
    return np.load(os.path.join(d, "out.npy"))


def kernel(M, x):
    M = np.ascontiguousarray(np.asarray(M, dtype=np.float32))
    x = np.ascontiguousarray(np.asarray(x, dtype=np.float32))
    assert M.shape == (N, N) and x.shape == (N, D)
    try:
        return _run_on_device(M, x)
    except Exception as e:
        print(f"kernel: in-process run failed ({e!r}); retrying in subprocess",
              file=sys.stderr, flush=True)
    last = None
    for _ in range(2):
        try:
            return _run_in_subprocess(M, x)
        except Exception as e:  # noqa: PERF203
            last = e
    raise last



# revision 8
# speedup vs baseline: 1.0827x; 1.0827x over previous
"""Trainium2 Bass kernel for out = exp(-M) @ x.

M: [16384, 16384] fp32, x: [16384, 128] fp32 -> out: [16384, 128] fp32.

Sharding: row-shard M and out over 8 cores (2048 rows each), x replicated.

Per-core pipeline (DMA-bound at ~143 MB HBM traffic, ~360 GB/s/core):
  DMA   : M tiles [128, K_WIN] fp32, natural layout (16 KiB contiguous rows),
          alternating the two HWDGE queues (SP / ACT sequencers)
  DMA   : x loaded in a few large strided transfers (512 B runs), either
          SWDGE with inline fp32->bf16 cast or HWDGE + DVE convert
  ACT   : e = exp(-M_tile) fused fp32 -> bf16 (free affine scale=-1)
  PE    : transpose e chunks [128m, 128k] -> PSUM [128k, 128m] (bf16)
  DVE   : evacuate PSUM -> SBUF rhs tiles [128k, 512m]
  PE    : out.T[f, m] += x[kchunk].T @ rhs   (x stationary bf16, fp32 acc)
  PE/DVE: final [f, m] -> [m, f] transpose, store via SWDGE (keeps the
          HWDGE queues free for the M stream)
  Tail  : the last super's K-windows taper (4096 ... 128) so the post-DMA
          pipeline drain is a few us instead of ~25 us.
"""

import os
import sys

sys.path.insert(0, "/opt/trn_rl_repo")

import numpy as np

import concourse.bass as bass  # noqa: F401  (engine namespaces live on nc)
import concourse.mybir as mybir
import concourse.tile as tile
from concourse import bacc
from concourse.bass_utils import run_bass_kernel_spmd
from concourse.masks import make_identity

N = 16384  # M is [N, N]
D = 128  # x is [N, D]
N_CORES = 8
M_ROWS = N // N_CORES  # 2048 rows of M / out per core

F32 = mybir.dt.float32
BF16 = mybir.dt.bfloat16
EXP = mybir.ActivationFunctionType.Exp

# geometry
M_SUPER = 512  # output rows accumulated per PSUM bank
N_SUPERS = M_ROWS // M_SUPER  # 4
K_WIN = int(os.environ.get("KWIN", "4096"))  # contraction window per M DMA tile
M_SUBS = M_SUPER // 128  # 4 m-subtiles per super
N_KCHUNKS = N // 128  # 128 total k-chunks

BUFS_M = int(os.environ.get("BUFS_M", "4"))
BUFS_E = int(os.environ.get("BUFS_E", "6"))
BUFS_PT = int(os.environ.get("BUFS_PT", "4"))
BUFS_RHS = int(os.environ.get("BUFS_RHS", "6"))
BUFS_PFIN = int(os.environ.get("BUFS_PFIN", "2"))
X_MODE = os.environ.get("X_MODE", "swdge")  # swdge | hwdge
X_PIECES = int(os.environ.get("X_PIECES", "4"))
TAPER = os.environ.get("TAPER", "1") == "1"


TAIL_SLICE = int(os.environ.get("TAIL_SLICE", "1024"))  # last-window piece width


def build_kernel(repeats=1, mode="full"):
    nc = bacc.Bacc("TRN2", target_bir_lowering=False, debug=False)
    m_t = nc.dram_tensor("m_shard", [M_ROWS, N], F32, kind="ExternalInput")
    x_t = nc.dram_tensor("x", [N, D], F32, kind="ExternalInput")
    out_t = nc.dram_tensor("out", [M_ROWS, D], F32, kind="ExternalOutput")
    m_ap = m_t.ap()
    out_ap = out_t.ap()
    # x viewed as [p, c, f]: element = x[c*128 + p, f]; partition stride 512 B,
    # c stride 64 KiB, f contiguous 512 B runs
    x_pcf = x_t.rearrange("(c p) f -> p c f", p=128)

    from contextlib import ExitStack

    with tile.TileContext(nc) as tc, ExitStack() as ctx:
        if repeats > 1:
            ctx.enter_context(tc.For_i(0, repeats, 1))
        consts = ctx.enter_context(tc.tile_pool(name="consts", bufs=1))
        ident_bf = consts.tile([128, 128], BF16)
        make_identity(nc, ident_bf[:])
        ident_f32 = consts.tile([128, 128], F32)
        make_identity(nc, ident_f32[:])

        # x resident in SBUF as bf16, chunk c at xbf[:, c*D:(c+1)*D]
        # (partition = k within chunk, free = feature).
        xbf_t = consts.tile([128, N_KCHUNKS * D], BF16)
        cper = N_KCHUNKS // X_PIECES
        if X_MODE == "swdge":
            # SWDGE casts fp32->bf16 inline; no staging, no DVE work
            for i in range(X_PIECES):
                nc.gpsimd.dma_start(
                    out=xbf_t[:, i * cper * D : (i + 1) * cper * D],
                    in_=x_pcf[:, i * cper : (i + 1) * cper, :],
                )
        else:
            xstage = ctx.enter_context(tc.tile_pool(name="xstage", bufs=2))
            for i in range(X_PIECES):
                xs = xstage.tile([128, cper * D], F32)
                eng = nc.sync if i % 2 == 0 else nc.scalar
                eng.dma_start(out=xs[:], in_=x_pcf[:, i * cper : (i + 1) * cper, :])
                nc.vector.tensor_copy(
                    xbf_t[:, i * cper * D : (i + 1) * cper * D], xs[:]
                )

        m_pool = ctx.enter_context(tc.tile_pool(name="m", bufs=BUFS_M))
        e_pool = ctx.enter_context(tc.tile_pool(name="e", bufs=BUFS_E))
        rhs_pool = ctx.enter_context(tc.tile_pool(name="rhs", bufs=BUFS_RHS))
        outT_pool = ctx.enter_context(tc.tile_pool(name="outT", bufs=2))
        outf_pool = ctx.enter_context(tc.tile_pool(name="outf", bufs=2))
        pt_pool = ctx.enter_context(tc.tile_pool(name="pt", bufs=BUFS_PT, space="PSUM"))
        pout_pool = ctx.enter_context(tc.tile_pool(name="pout", bufs=2, space="PSUM"))
        pfin_pool = ctx.enter_context(
            tc.tile_pool(name="pfin", bufs=BUFS_PFIN, space="PSUM")
        )

        dma_idx = 0  # alternates the two HWDGE queues across all M tiles
        n_wins = N // K_WIN
        for ms in range(N_SUPERS):
            pout = (
                pout_pool.tile([128, M_SUPER], F32, name="pout", tag="pout")
                if mode not in ("mem", "dma")
                else None
            )
            outT_mem = (
                outT_pool.tile([128, M_SUPER], F32, name="outT", tag="outT")
                if mode in ("mem", "dma")
                else None
            )
            c0 = 0  # column offset of current window
            for kw in range(n_wins):
                w = K_WIN
                # last window of the run: slice DMA + exp so the post-DMA
                # drain is one small slice deep, not a whole window
                last = TAPER and ms == N_SUPERS - 1 and kw == n_wins - 1
                ws = TAIL_SLICE if last else w
                n_sl = w // ws
                mts, ebf = [], []
                for j in range(M_SUBS):
                    mts.append(m_pool.tile([128, w], F32, name=f"mt{j}", tag="mt"))
                    if mode != "dma":
                        ebf.append(
                            e_pool.tile([128, w], BF16, name=f"e{j}", tag="e")
                        )
                for s in range(n_sl):
                    for j in range(M_SUBS):
                        mt = mts[j]
                        r0 = ms * M_SUPER + j * 128
                        dma_eng = nc.sync if dma_idx % 2 == 0 else nc.scalar
                        dma_idx += 1
                        dma_eng.dma_start(
                            out=mt[:, s * ws : (s + 1) * ws],
                            in_=m_ap[r0 : r0 + 128, c0 + s * ws : c0 + (s + 1) * ws],
                        )
                        if mode == "dma":
                            if s == 0:
                                nc.vector.tensor_copy(
                                    outT_mem[:, j * 128 : (j + 1) * 128], mt[:, 0:128]
                                )
                            continue
                        nc.scalar.activation(
                            ebf[j][:, s * ws : (s + 1) * ws],
                            mt[:, s * ws : (s + 1) * ws],
                            EXP,
                            scale=-1.0,
                        )
                if mode == "dma":
                    c0 += w
                    continue
                if mode == "mem":
                    for j in range(M_SUBS):
                        nc.vector.tensor_copy(
                            outT_mem[:, j * 128 : (j + 1) * 128], ebf[j][:, 0:128]
                        )
                    c0 += w
                    continue
                for kc in range(w // 128):
                    kgg = (c0 // 128) + kc
                    if mode == "noT":
                        off = min(kc * 128, w - M_SUPER) if w >= M_SUPER else 0
                        nc.tensor.matmul(
                            pout[:],
                            lhsT=xbf_t[:, kgg * D : (kgg + 1) * D],
                            rhs=ebf[0][:, off : off + M_SUPER],
                            start=(kgg == 0),
                            stop=(kgg == N_KCHUNKS - 1),
                        )
                        continue
                    pt = pt_pool.tile([128, M_SUPER], BF16)
                    for j in range(M_SUBS):
                        nc.tensor.transpose(
                            pt[:, j * 128 : (j + 1) * 128],
                            ebf[j][:, kc * 128 : (kc + 1) * 128],
                            ident_bf[:],
                        )
                    rhs = rhs_pool.tile([128, M_SUPER], BF16)
                    nc.vector.tensor_copy(rhs[:], pt[:])
                    nc.tensor.matmul(
                        pout[:],
                        lhsT=xbf_t[:, kgg * D : (kgg + 1) * D],
                        rhs=rhs[:],
                        start=(kgg == 0),
                        stop=(kgg == N_KCHUNKS - 1),
                    )
                c0 += w
            # evacuate out.T [f, m] and transpose to [m, f]
            if mode in ("mem", "dma"):
                outT = outT_mem
            else:
                outT = outT_pool.tile([128, M_SUPER], F32)
                nc.vector.tensor_copy(outT[:], pout[:])
            for j in range(M_SUBS):
                pf = pfin_pool.tile([128, D], F32)
                nc.tensor.transpose(
                    pf[:], outT[:, j * 128 : (j + 1) * 128], ident_f32[:]
                )
                of = outf_pool.tile([128, D], F32)
                nc.vector.tensor_copy(of[:], pf[:])
                r0 = ms * M_SUPER + j * 128
                nc.gpsimd.dma_start(out=out_ap[r0 : r0 + 128, :], in_=of[:])

    nc.compile()
    return nc


_NC_CACHE = None


def _get_nc():
    global _NC_CACHE
    if _NC_CACHE is None:
        _NC_CACHE = build_kernel()
    return _NC_CACHE


def _run_on_device(M, x):
    nc = _get_nc()
    in_maps = [
        {"m_shard": M[c * M_ROWS : (c + 1) * M_ROWS], "x": x} for c in range(N_CORES)
    ]
    res = run_bass_kernel_spmd(nc, in_maps, list(range(N_CORES)))
    return np.concatenate([res.results[c]["out"] for c in range(N_CORES)], axis=0)


def _run_in_subprocess(M, x):
    """Retry path: a fresh process gets a fresh NRT/axon session, which
    recovers from the occasional NRT_EXEC_UNIT_UNRECOVERABLE flake."""
    import subprocess, tempfile

    d = tempfile.mkdtemp(prefix="bassk_")
    np.save(os.path.join(d, "M.npy"), M)
    np.save(os.path.join(d, "x.npy"), x)
    here = os.path.dirname(os.path.abspath(__file__))
    code = (
        "import sys, numpy as np\n"
        f"sys.path.insert(0, {here!r})\n"
        "import kernel\n"
        f"M = np.load({os.path.join(d, 'M.npy')!r})\n"
        f"x = np.load({os.path.join(d, 'x.npy')!r})\n"
        "out = kernel._run_on_device(M, x)\n"
        f"np.save({os.path.join(d, 'out.npy')!r}, out)\n"
    )
    subprocess.run([sys.executable, "-c", code], check=True, timeout=1200)
    return np.load(os.path.join(d, "out.npy"))


def kernel(M, x):
    M = np.ascontiguousarray(np.asarray(M, dtype=np.float32))
    x = np.ascontiguousarray(np.asarray(x, dtype=np.float32))
    assert M.shape == (N, N) and x.shape == (N, D)
    try:
        return _run_on_device(M, x)
    except Exception as e:
        print(f"kernel: in-process run failed ({e!r}); retrying in subprocess",
              file=sys.stderr, flush=True)
    last = None
    for _ in range(2):
        try:
            return _run_in_subprocess(M, x)
        except Exception as e:  # noqa: PERF203
            last = e
    raise last


# revision 15
# speedup vs baseline: 1.2237x; 1.1303x over previous
"""Trainium2 Bass kernel for out = exp(-M) @ x.

M: [16384, 16384] fp32, x: [16384, 128] fp32 -> out: [16384, 128] fp32.

Sharding: row-shard M and out over 8 cores (2048 rows each), x replicated.

Per-core pipeline (DMA-bound at ~143 MB HBM traffic, ~360 GB/s/core):
  DMA   : M tiles [128, K_WIN] fp32, natural layout (16 KiB contiguous rows),
          alternating the two HWDGE queues (SP / ACT sequencers)
  DMA   : x loaded in a few large strided transfers (512 B runs), either
          SWDGE with inline fp32->bf16 cast or HWDGE + DVE convert
  ACT   : e = exp(-M_tile) fused fp32 -> bf16 (free affine scale=-1)
  PE    : transpose e chunks [128m, 128k] -> PSUM [128k, 128m] (bf16)
  DVE   : evacuate PSUM -> SBUF rhs tiles [128k, 512m]
  PE    : out.T[f, m] += x[kchunk].T @ rhs   (x stationary bf16, fp32 acc)
  PE/DVE: final [f, m] -> [m, f] transpose, store via SWDGE (keeps the
          HWDGE queues free for the M stream)
  Tail  : the last super's K-windows taper (4096 ... 128) so the post-DMA
          pipeline drain is a few us instead of ~25 us.
"""

import os
import sys

sys.path.insert(0, "/opt/trn_rl_repo")

import numpy as np

import concourse.bass as bass  # noqa: F401  (engine namespaces live on nc)
import concourse.mybir as mybir
import concourse.tile as tile
from concourse import bacc
from concourse.bass_utils import run_bass_kernel_spmd
from concourse.masks import make_identity

N = 16384  # M is [N, N]
D = 128  # x is [N, D]
N_CORES = 8
M_ROWS = N // N_CORES  # 2048 rows of M / out per core

F32 = mybir.dt.float32
BF16 = mybir.dt.bfloat16
EXP = mybir.ActivationFunctionType.Exp

# geometry
M_SUPER = 512  # output rows accumulated per PSUM bank
N_SUPERS = M_ROWS // M_SUPER  # 4
K_WIN = int(os.environ.get("KWIN", "4096"))  # contraction window per M DMA tile
M_SUBS = M_SUPER // 128  # 4 m-subtiles per super
N_KCHUNKS = N // 128  # 128 total k-chunks

BUFS_M = int(os.environ.get("BUFS_M", "6"))
BUFS_E = int(os.environ.get("BUFS_E", "7"))
BUFS_PT = int(os.environ.get("BUFS_PT", "4"))
BUFS_RHS = int(os.environ.get("BUFS_RHS", "6"))
BUFS_PFIN = int(os.environ.get("BUFS_PFIN", "2"))
X_MODE = os.environ.get("X_MODE", "swdge")  # swdge | hwdge
X_PIECES = int(os.environ.get("X_PIECES", "4"))
TAPER = os.environ.get("TAPER", "1") == "1"
# M-tile DMA issue queues: the ACT sequencer (nc.scalar) runs the exps, so
# dma_starts queued there stall behind 3.7us activation instructions in the
# strict 8-deep FIFO. Default: SP only.
DMA_MIX = os.environ.get("DMA_MIX", "sync")  # sync | hwsw | 2hw | 3way


TAIL_SLICE = int(os.environ.get("TAIL_SLICE", "512"))  # last-window piece width
TAIL2_SLICE = int(os.environ.get("TAIL2_SLICE", "1024"))  # 2nd-to-last window piece


def build_kernel(repeats=1, mode="full"):
    nc = bacc.Bacc("TRN2", target_bir_lowering=False, debug=False)
    m_t = nc.dram_tensor("m_shard", [M_ROWS, N], F32, kind="ExternalInput")
    x_t = nc.dram_tensor("x", [N, D], F32, kind="ExternalInput")
    out_t = nc.dram_tensor("out", [M_ROWS, D], F32, kind="ExternalOutput")
    m_ap = m_t.ap()
    out_ap = out_t.ap()
    # x viewed as [p, c, f]: element = x[c*128 + p, f]; partition stride 512 B,
    # c stride 64 KiB, f contiguous 512 B runs
    x_pcf = x_t.rearrange("(c p) f -> p c f", p=128)

    from contextlib import ExitStack

    with tile.TileContext(nc) as tc, ExitStack() as ctx:
        if repeats > 1:
            ctx.enter_context(tc.For_i(0, repeats, 1))
        consts = ctx.enter_context(tc.tile_pool(name="consts", bufs=1))
        ident_bf = consts.tile([128, 128], BF16)
        make_identity(nc, ident_bf[:])
        ident_f32 = consts.tile([128, 128], F32)
        make_identity(nc, ident_f32[:])

        # x resident in SBUF as bf16, chunk c at xbf[:, c*D:(c+1)*D]
        # (partition = k within chunk, free = feature).
        xbf_t = consts.tile([128, N_KCHUNKS * D], BF16)
        cper = N_KCHUNKS // X_PIECES
        if os.environ.get("X_SKIP") and mode == "dma":
            pass  # probe: no x load at all (only valid when PE never reads it)
        elif X_MODE == "swdge":
            # SWDGE casts fp32->bf16 inline; no staging, no DVE work
            for i in range(X_PIECES):
                nc.gpsimd.dma_start(
                    out=xbf_t[:, i * cper * D : (i + 1) * cper * D],
                    in_=x_pcf[:, i * cper : (i + 1) * cper, :],
                )
        else:
            xstage = ctx.enter_context(tc.tile_pool(name="xstage", bufs=2))
            for i in range(X_PIECES):
                xs = xstage.tile([128, cper * D], F32)
                eng = nc.sync if i % 2 == 0 else nc.scalar
                eng.dma_start(out=xs[:], in_=x_pcf[:, i * cper : (i + 1) * cper, :])
                nc.vector.tensor_copy(
                    xbf_t[:, i * cper * D : (i + 1) * cper * D], xs[:]
                )

        m_pool = ctx.enter_context(tc.tile_pool(name="m", bufs=BUFS_M))
        e_pool = ctx.enter_context(tc.tile_pool(name="e", bufs=BUFS_E))
        rhs_pool = ctx.enter_context(tc.tile_pool(name="rhs", bufs=BUFS_RHS))
        outT_pool = ctx.enter_context(tc.tile_pool(name="outT", bufs=2))
        outf_pool = ctx.enter_context(tc.tile_pool(name="outf", bufs=2))
        pt_pool = ctx.enter_context(tc.tile_pool(name="pt", bufs=BUFS_PT, space="PSUM"))
        pout_pool = ctx.enter_context(tc.tile_pool(name="pout", bufs=2, space="PSUM"))
        pfin_pool = ctx.enter_context(
            tc.tile_pool(name="pfin", bufs=BUFS_PFIN, space="PSUM")
        )

        dma_idx = 0  # alternates the two HWDGE queues across all M tiles
        n_wins = N // K_WIN
        for ms in range(N_SUPERS):
            pout = (
                pout_pool.tile([128, M_SUPER], F32, name="pout", tag="pout")
                if mode not in ("mem", "dma")
                else None
            )
            outT_mem = (
                outT_pool.tile([128, M_SUPER], F32, name="outT", tag="outT")
                if mode in ("mem", "dma")
                else None
            )
            c0 = 0  # column offset of current window
            for kw in range(n_wins):
                w = K_WIN
                # final windows of the run: slice DMA + exp so the post-DMA
                # drain is one small slice deep, not a whole window (the
                # second-to-last window is sliced coarser so ACT enters the
                # last window nearly caught up)
                last = TAPER and ms == N_SUPERS - 1 and kw == n_wins - 1
                last2 = TAPER and ms == N_SUPERS - 1 and kw == n_wins - 2
                ws = TAIL_SLICE if last else (TAIL2_SLICE if last2 else w)
                n_sl = w // ws
                mts, ebf = [], []
                for j in range(M_SUBS):
                    mts.append(m_pool.tile([128, w], F32, name=f"mt{j}", tag="mt"))
                    if mode != "dma":
                        ebf.append(
                            e_pool.tile([128, w], BF16, name=f"e{j}", tag="e")
                        )
                for s in range(n_sl):
                    for j in range(M_SUBS):
                        mt = mts[j]
                        r0 = ms * M_SUPER + j * 128
                        if DMA_MIX == "sync":
                            dma_eng = nc.sync
                        elif DMA_MIX == "hwsw":
                            dma_eng = nc.sync if dma_idx % 2 == 0 else nc.gpsimd
                        elif DMA_MIX == "3way":
                            dma_eng = (nc.sync, nc.scalar, nc.gpsimd)[dma_idx % 3]
                        else:  # 2hw: the original sync/scalar alternation
                            dma_eng = nc.sync if dma_idx % 2 == 0 else nc.scalar
                        dma_idx += 1
                        dma_eng.dma_start(
                            out=mt[:, s * ws : (s + 1) * ws],
                            in_=m_ap[r0 : r0 + 128, c0 + s * ws : c0 + (s + 1) * ws],
                        )
                        if mode == "dma":
                            if s == 0:
                                nc.vector.tensor_copy(
                                    outT_mem[:, j * 128 : (j + 1) * 128], mt[:, 0:128]
                                )
                            continue
                        nc.scalar.activation(
                            ebf[j][:, s * ws : (s + 1) * ws],
                            mt[:, s * ws : (s + 1) * ws],
                            EXP,
                            scale=-1.0,
                        )
                if mode == "dma":
                    c0 += w
                    continue
                if mode == "mem":
                    for j in range(M_SUBS):
                        nc.vector.tensor_copy(
                            outT_mem[:, j * 128 : (j + 1) * 128], ebf[j][:, 0:128]
                        )
                    c0 += w
                    continue
                for kc in range(w // 128):
                    kgg = (c0 // 128) + kc
                    if mode == "noT":
                        off = min(kc * 128, w - M_SUPER) if w >= M_SUPER else 0
                        nc.tensor.matmul(
                            pout[:],
                            lhsT=xbf_t[:, kgg * D : (kgg + 1) * D],
                            rhs=ebf[0][:, off : off + M_SUPER],
                            start=(kgg == 0),
                            stop=(kgg == N_KCHUNKS - 1),
                        )
                        continue
                    pt = pt_pool.tile([128, M_SUPER], BF16)
                    for j in range(M_SUBS):
                        nc.tensor.transpose(
                            pt[:, j * 128 : (j + 1) * 128],
                            ebf[j][:, kc * 128 : (kc + 1) * 128],
                            ident_bf[:],
                        )
                    rhs = rhs_pool.tile([128, M_SUPER], BF16)
                    nc.vector.tensor_copy(rhs[:], pt[:])
                    nc.tensor.matmul(
                        pout[:],
                        lhsT=xbf_t[:, kgg * D : (kgg + 1) * D],
                        rhs=rhs[:],
                        start=(kgg == 0),
                        stop=(kgg == N_KCHUNKS - 1),
                    )
                c0 += w
            # evacuate out.T [f, m] and transpose to [m, f]
            if mode in ("mem", "dma"):
                outT = outT_mem
            else:
                outT = outT_pool.tile([128, M_SUPER], F32)
                nc.vector.tensor_copy(outT[:], pout[:])
            for j in range(M_SUBS):
                pf = pfin_pool.tile([128, D], F32)
                nc.tensor.transpose(
                    pf[:], outT[:, j * 128 : (j + 1) * 128], ident_f32[:]
                )
                of = outf_pool.tile([128, D], F32)
                nc.vector.tensor_copy(of[:], pf[:])
                r0 = ms * M_SUPER + j * 128
                o_eng = {"sync": nc.sync, "scalar": nc.scalar}.get(
                    os.environ.get("OUT_ENG", "gpsimd"), nc.gpsimd
                )
                if ms == N_SUPERS - 1 and os.environ.get("OUT_LAST", "scalar") == "scalar":
                    # by the last super's drain the ACT queue is empty; its
                    # HWDGE stores skip SWDGE's ~1us Q7 emission latency
                    o_eng = nc.scalar
                o_eng.dma_start(out=out_ap[r0 : r0 + 128, :], in_=of[:])

    nc.compile()
    return nc


_NC_CACHE = None


def _get_nc():
    global _NC_CACHE
    if _NC_CACHE is None:
        _NC_CACHE = build_kernel()
    return _NC_CACHE


def _run_on_device(M, x):
    nc = _get_nc()
    in_maps = [
        {"m_shard": M[c * M_ROWS : (c + 1) * M_ROWS], "x": x} for c in range(N_CORES)
    ]
    res = run_bass_kernel_spmd(nc, in_maps, list(range(N_CORES)))
    return np.concatenate([res.results[c]["out"] for c in range(N_CORES)], axis=0)


def _run_in_subprocess(M, x):
    """Retry path: a fresh process gets a fresh NRT/axon session, which
    recovers from the occasional NRT_EXEC_UNIT_UNRECOVERABLE flake."""
    import subprocess, tempfile

    d = tempfile.mkdtemp(prefix="bassk_")
    np.save(os.path.join(d, "M.npy"), M)
    np.save(os.path.join(d, "x.npy"), x)
    here = os.path.dirname(os.path.abspath(__file__))
    code = (
        "import sys, numpy as np\n"
        f"sys.path.insert(0, {here!r})\n"
        "import kernel\n"
        f"M = np.load({os.path.join(d, 'M.npy')!r})\n"
        f"x = np.load({os.path.join(d, 'x.npy')!r})\n"
        "out = kernel._run_on_device(M, x)\n"
        f"np.save({os.path.join(d, 'out.npy')!r}, out)\n"
    )
    subprocess.run([sys.executable, "-c", code], check=True, timeout=1200)
    return np.load(os.path.join(d, "out.npy"))


def kernel(M, x):
    M = np.ascontiguousarray(np.asarray(M, dtype=np.float32))
    x = np.ascontiguousarray(np.asarray(x, dtype=np.float32))
    assert M.shape == (N, N) and x.shape == (N, D)
    try:
        return _run_on_device(M, x)
    except Exception as e:
        print(f"kernel: in-process run failed ({e!r}); retrying in subprocess",
              file=sys.stderr, flush=True)
    last = None
    for _ in range(2):
        try:
            return _run_in_subprocess(M, x)
        except Exception as e:  # noqa: PERF203
            last = e
    raise last
